# revision 20
# baseline (speedup 1.0000x reference)
"""Trainium2 Bass kernel for the fused attention block:

    qkv = x @ w_qkv ; q,k,v split; heads; dots = q @ k.reshape(bh, D, n)
    attn = softmax(dots); out = attn @ v; merge heads; out = out @ w_out + b_out
    out = LayerNorm(out) * ln_g + ln_b; return out + x

Sharding: data-parallel over batch b (8 batches -> 8 NeuronCores, weights
replicated). Each core runs an identical program on its own batch slice.

Key layout choices (per core, N=1024 seq, DIM=512, H=8 heads, D=64):
  - xT [512, 1024] via matmul-transposes (x chunk stationary, identity
    moving): a REGULAR full-array matmul registers as HAM activity, so
    phase 1 doubles as the PE clock-gate warmup. x is DMA'd in 8 row
    chunks so the transposes chase the DMA.
  - qT [512, 1024]  = matmul(lhsT=w_q, rhs=xT), rounded to bf16 at psum
    evacuation (the dots operands are bf16 to halve the k DMA round trip).
  - k is round-tripped through a DRAM scratch (bf16) so the faithful
    k.reshape(D, n) ("k_r") can be gathered as [64, 1024] with d on
    partitions; the other parity's 64 rows stay zero so the K=128 dots
    contraction nulls the other head's rows of the shared qT pair tile.
    (Half-array K=64 matmuls are NOT used: they neither register as HAM
    activity nor run at the warm clock.)
  - exp: both heads on ACT as single [128, 1024] instructions -- psum
    reads bottleneck exp, and a 2-psum-bank tile reads twice as fast as
    a 1-bank tile, so never split exp below 1024 columns.
  - out_hT[e, i] += matmul(lhsT=[v|ones] block, rhs=expT); the ones column
    makes the same accumulation chain produce the softmax denominator S[i].
    Garbage in the unused output partitions is never read.
  - normalize with a partition-parallel reciprocal + DRAM-broadcast of 1/S;
    pav evacuations copy the full 128 partitions (partial-partition DVE ops
    run at proportionally reduced rate).
  - final = matmul(lhsT=out_catT, rhs=w_out) -> LN (bn_stats/bn_aggr,
    rsqrt via exp(-0.5*ln(var+eps)) to stay in one ACT table set) +
    residual, with the residual add split across DVE and gpsimd.
"""

import os
import numpy as np

B, N, DIM = 8, 1024, 512
H, D = 8, 64
LN_EPS = 1e-5
N_CORES = 8

# Schraudolph exp in bf16: bitcast(int16(x*SCH_A + SCH_B)) ~= exp(x).
# The -5.5 recentres the piecewise-linear 2^frac error to +-3.2%.
SCH_A = 128.0 * np.log2(np.e)
SCH_B = 127.0 * 128.0 - 5.5

_cache = {}
last_results = None


MAX_WAITS = 1


def _split_sync_waits(nc, limit=MAX_WAITS):
    """This walrus build rejects instructions carrying more than `limit`
    sem-wait commands ("Too many sync wait commands"). Move excess waits
    onto same-engine NOPs inserted immediately before the instruction
    (per-engine program order is list order, so semantics are identical)."""
    import concourse.mybir as mybir

    for fn in nc.m.functions:
        for bb in fn.blocks:
            out = []
            for ins in bb.instructions:
                si = getattr(ins, "sync_info", None)
                keep = 0 if type(ins).__name__ in ("InstISA", "InstDrain") else limit
                if si is not None and si.on_wait and len(si.on_wait) > keep:
                    waits = list(si.on_wait)
                    si.on_wait = waits[len(waits) - keep :] if keep else []
                    extra = waits[: len(waits) - keep]
                    for i in range(0, len(extra), limit):
                        out.append(
                            mybir.InstNoOp(
                                name=f"{ins.name}_w{i}",
                                engine=ins.engine,
                                debug=ins.debug,
                                bass_nofuse=True,
                                sync_info=mybir.SyncInfo(
                                    on_wait=extra[i : i + limit], on_update=[]
                                ),
                            )
                        )
                out.append(ins)
            bb.instructions = out


def _patch_sem_clear():
    """EVENT_SEMAPHORE_RANGE_CLEAR with a large sem range fails walrus
    codegen ("ISA wrong length"); chunk the tail sem clear into <=48-sem
    ranges (the size known to compile)."""
    import concourse.bass as bass
    from concourse.bass import SemaphoreHandle

    if getattr(bass.Bass, "_sem_clear_patched", False):
        return
    from concourse.bass import compact_to_ranges

    def clear_and_free_semaphores(self, sems):
        if not sems:
            return
        sem_nums = [s.num if isinstance(s, SemaphoreHandle) else s for s in sems]
        for sem_range in compact_to_ranges(sem_nums):
            for lo in range(sem_range.start, sem_range.stop, 48):
                sub = range(lo, min(lo + 48, sem_range.stop))
                assert self._state.free_isdisjoint(sub)
                self.gpsimd.dma_reset(sub)
                self.gpsimd.sem_clear(sub)
        self._state.prepend_free_semaphores(sem_nums)
        for poison_set in self._tile_sem_poison_stack:
            poison_set.update(sem_nums)

    bass.Bass.clear_and_free_semaphores = clear_and_free_semaphores
    bass.Bass._sem_clear_patched = True

    import concourse.tile as tile
    from concourse.vector_clock import ScopedClock

    def _drain_and_barrier(self, tick_clock, wait_clock):
        drain_inst = self.nc.sync.drain()
        wait_clock.add_sem_waits(
            drain_inst.ins, ScopedClock({None: tick_clock.global_clock})
        )
        self.nc.all_engine_barrier()
        popped = self.nc._tile_sem_poison_stack.pop()
        assert popped is self._sem_poison
        self.nc.clear_and_free_semaphores(list(self.sems.allocated().values()))

    tile.TileContext._drain_and_barrier = _drain_and_barrier


def _build(trivial_bias: bool, trivial_gamma: bool, trivial_beta: bool):
    import concourse.bass as bass
    import concourse.mybir as mybir
    import concourse.tile as tile
    from concourse.masks import make_identity

    _patch_sem_clear()

    fp32 = mybir.dt.float32
    fp32r = mybir.dt.float32r
    bf16 = mybir.dt.bfloat16
    i16 = mybir.dt.int16
    AF = mybir.ActivationFunctionType
    ALU = mybir.AluOpType

    nc = bass.Bass("TRN2", target_bir_lowering=False, debug=False)

    x_d = nc.dram_tensor("x", [N, DIM], fp32, kind="ExternalInput")
    wqkv_d = nc.dram_tensor("w_qkv", [DIM, 3 * DIM], fp32r, kind="ExternalInput")
    wout_d = nc.dram_tensor("w_out", [DIM, DIM], fp32, kind="ExternalInput")
    bout_d = nc.dram_tensor("b_out", [1, DIM], fp32, kind="ExternalInput")
    lng_d = nc.dram_tensor("ln_g", [1, DIM], fp32, kind="ExternalInput")
    lnb_d = nc.dram_tensor("ln_b", [1, DIM], fp32, kind="ExternalInput")
    out_d = nc.dram_tensor("out", [N, DIM], fp32, kind="ExternalOutput")

    NT = N // 128      # 8 i-tiles (also c-tiles)
    KC = DIM // 128    # 4 contraction chunks

    with tile.TileContext(nc) as tc:
        import contextlib

        ctx = contextlib.ExitStack()
        with ctx:
            singles = ctx.enter_context(tc.tile_pool(name="singles", bufs=1))
            dram = ctx.enter_context(tc.tile_pool(name="dram", bufs=1, space="DRAM"))
            # 2x [128,1024] general psum slots + 2x [128,1024] AV slots
            ps_sm = ctx.enter_context(tc.tile_pool(name="ps_sm", bufs=2, space="PSUM"))
            ps_av = ctx.enter_context(tc.tile_pool(name="ps_av", bufs=2, space="PSUM"))
            temps = ctx.enter_context(tc.tile_pool(name="temps", bufs=4))
            exps = ctx.enter_context(tc.tile_pool(name="exps", bufs=4))
            lnp = ctx.enter_context(tc.tile_pool(name="lnp", bufs=6))

            # ---- constants
            ident_bf = singles.tile([128, 128], bf16)
            make_identity(nc, ident_bf)
            eps_sb = singles.tile([128, 1], fp32)
            nc.vector.memset(eps_sb, LN_EPS)

            # ---- input loads. x first (needed from ~1.5us), chunked per
            # m-tile so ph1 transposes can chase the DMA; then wq (needed
            # ~8us), wk, wv. Small/late tensors go on the gpsimd queue.
            x_sb = singles.tile([128, NT, DIM], fp32)  # x[128*m + p, c]
            xb_sb = singles.tile([128, NT, DIM], bf16)  # bf16 copy for ph1
            for m in range(NT):
                nc.gpsimd.dma_start(
                    out=xb_sb[:, m, :], in_=x_d.ap()[m * 128 : (m + 1) * 128, :]
                )
            wq_sb = singles.tile([128, KC, DIM], fp32r)
            wk_sb = singles.tile([128, KC, DIM], fp32r)
            wv_sb = singles.tile([128, KC, DIM], fp32r)
            for w_sb, lo in ((wq_sb, 0), (wk_sb, DIM), (wv_sb, 2 * DIM)):
                for kc in range(KC):
                    nc.sync.dma_start(
                        out=w_sb[:, kc, :],
                        in_=wqkv_d.ap()[kc * 128 : (kc + 1) * 128, lo : lo + DIM],
                    )
            # fp32 x for the residual: needed only by the projection phase,
            # so it loads after everything else on the sync queue
            for m in range(NT):
                nc.sync.dma_start(
                    out=x_sb[:, m, :], in_=x_d.ap()[m * 128 : (m + 1) * 128, :]
                )
            # w_out stored per head PAIR ([128, 4, 512]) so the projection
            # contracts K=128 (full array).
            wout_sb = singles.tile([128, H // 2, DIM], bf16)
            nc.gpsimd.dma_start(
                out=wout_sb, in_=wout_d.ap().rearrange("(p r) f -> r p f", r=128)
            )

            bb_sb = gb_sb = bb2_sb = None
            if not trivial_bias:
                bb_sb = singles.tile([128, DIM], fp32)
                nc.gpsimd.dma_start(
                    out=bb_sb,
                    in_=bass.AP(tensor=bout_d, offset=0, ap=[[0, 128], [1, DIM]]),
                )
            if not trivial_gamma:
                gb_sb = singles.tile([128, DIM], fp32)
                nc.gpsimd.dma_start(
                    out=gb_sb,
                    in_=bass.AP(tensor=lng_d, offset=0, ap=[[0, 128], [1, DIM]]),
                )
            if not trivial_beta:
                bb2_sb = singles.tile([128, DIM], fp32)
                nc.gpsimd.dma_start(
                    out=bb2_sb,
                    in_=bass.AP(tensor=lnb_d, offset=0, ap=[[0, 128], [1, DIM]]),
                )

            # ---- v blocks: [128, 128] lhsT per (tile, head). Even head ->
            # v in cols 0:64 + ones col 64 (AV output in psum rows 0:64, S in
            # row 64); odd head -> v in cols 64:128 + ones col 0 (output rows
            # 64:128, S row 0). The never-written filler columns produce
            # garbage only in psum rows that are never read, so only the ones
            # columns need initialising -- but memset everything once on the
            # (otherwise idle) gpsimd engine to keep NaNs out of the array.
            v_sb = singles.tile([128, NT, H, 128], bf16)
            nc.gpsimd.memset(v_sb, 0.0)
            v_par = v_sb.rearrange("p m (h2 par) c -> p m h2 par c", par=2)
            nc.gpsimd.memset(v_par[:, :, :, 0, D : D + 1], 1.0)
            nc.gpsimd.memset(v_par[:, :, :, 1, 0:1], 1.0)

            warm = singles.tile([128, 512], fp32r)
            nc.vector.memset(warm.bitcast(fp32), 1.0)
            for i in range(8):
                pw = ps_av.tile([128, 512], fp32, tag="av", name=f"pw{i}")
                nc.tensor.matmul(pw, warm[:, 0:128], warm, start=True, stop=True)

            # ---- phase 1: xT[k, i] via matmul-transposes, chasing the x
            # DMA: out = x_chunk.T @ I. A REGULAR full-array matmul (lhsT =
            # x chunk as stationary, identity as moving) -- unlike
            # transpose-mode this registers as HAM activity, so it doubles
            # as the clock-gate warmup and needs no junk-matmul fillers.
            xT_sb = singles.tile([128, KC, N], fp32r)
            for mp in range(NT // 2):
                pt = ps_sm.tile([128, 1024], fp32, tag="sm", name=f"pt{mp}")
                ptv = pt.rearrange("p (mh kc c) -> p mh kc c", mh=2, kc=KC)
                for mh in range(2):
                    m = 2 * mp + mh
                    for kc in range(KC):
                        nc.tensor.matmul(
                            ptv[:, mh, kc, :],
                            xb_sb[:, m, kc * 128 : (kc + 1) * 128],
                            ident_bf,
                            start=True,
                            stop=True,
                        )
                nc.scalar.copy(
                    out=xT_sb[:, :, 2 * mp * 128 : (2 * mp + 2) * 128].rearrange(
                        "p kc (mh c) -> p kc mh c", mh=2
                    ),
                    in_=ptv.rearrange("p mh kc c -> p kc mh c"),
                )

            # ---- phase 2: qT[qd, i], two heads per tile (M=128, full array).
            # qT is rounded to bf16 at psum evacuation: the row-tiled dots
            # pair can only stream both rhs operands concurrently at 2 bytes
            # per element (two fp32r streams saturate the moving-operand bus
            # and the pair runs at half rate).
            qT_sb = singles.tile([128, KC, N], bf16)
            for m in range(KC):
                pq = [
                    ps_sm.tile([128, 512], fp32, tag="sm", name=f"pq{m}_{nb}")
                    for nb in range(2)
                ]
                for kc in range(KC):
                    for nb in range(2):
                        nc.tensor.matmul(
                            pq[nb],
                            wq_sb[:, kc, m * 128 : (m + 1) * 128],
                            xT_sb[:, kc, nb * 512 : (nb + 1) * 512],
                            start=(kc == 0),
                            stop=(kc == KC - 1),
                        )
                for nb in range(2):
                    nc.vector.tensor_copy(
                        qT_sb[:, m, nb * 512 : (nb + 1) * 512], pq[nb]
                    )

            # ---- phase 3a: k natural -> DRAM scratch (so the faithful
            # k_r reshape can be gathered with d on partitions).
            k_dram = dram.tile([N, DIM], bf16)
            for m in range(NT):
                pk = ps_sm.tile([128, DIM], fp32, tag="sm", name=f"pk{m}")
                for kc in range(KC):
                    nc.tensor.matmul(
                        pk,
                        xT_sb[:, kc, m * 128 : (m + 1) * 128],
                        wk_sb[:, kc, :],
                        start=(kc == 0),
                        stop=(kc == KC - 1),
                    )
                ktmp = temps.tile([128, DIM], bf16, tag="ktmp")
                nc.vector.tensor_copy(ktmp, pk)
                nc.sync.dma_start(out=k_dram[m * 128 : (m + 1) * 128, :], in_=ktmp)

            # krr gathers: head h's k_r sits at its parity rows ((h%2)*64);
            # the other 64 rows stay ZERO so the K=128 dots contraction
            # nulls the other head's rows of the shared qT pair tile.
            krr_all = singles.tile([128, H, N], bf16)
            nc.gpsimd.memset(krr_all, 0.0)

            def load_krr(hp):
                for hh in (2 * hp, 2 * hp + 1):
                    r0 = (hh % 2) * 64
                    nc.gpsimd.dma_start(
                        out=krr_all[r0 : r0 + 64, hh, :].rearrange(
                            "p (s c) -> p s c", s=16
                        ),
                        in_=bass.AP(
                            tensor=k_dram.tensor,
                            offset=k_dram.offset + hh * 64,
                            ap=[[16 * DIM, 64], [DIM, 16], [1, 64]],
                        ),
                    )

            load_krr(0)
            load_krr(1)

            # ---- phase 3b: v half
            for m in range(NT):
                pvv = ps_sm.tile([128, DIM], fp32, tag="sm", name=f"pvv{m}")
                for kc in range(KC):
                    nc.tensor.matmul(
                        pvv,
                        xT_sb[:, kc, m * 128 : (m + 1) * 128],
                        wv_sb[:, kc, :],
                        start=(kc == 0),
                        stop=(kc == KC - 1),
                    )
                vv = v_sb[:, m, :, :].rearrange("p (h2 par) c -> p h2 par c", par=2)
                pv = pvv.rearrange("p (h2 par e) -> p h2 par e", h2=4, par=2)
                nc.vector.tensor_copy(vv[:, :, 0, 0:64], pv[:, :, 0, :])
                nc.vector.tensor_copy(vv[:, :, 1, 64:128], pv[:, :, 1, :])

            # ---- phase 4: attention.
            # Per (pair, ct): row-tiled dots (2 concurrent K=64 matmuls per
            # nb slot), exp split ACT/DVE, then the previous ct's AV pair.
            # PSUM: 4x dots [128,512] (ps_sm) + 2x pav [128,1024] (ps_av)
            # fills all 8 banks, so there are no fillers inside this phase.
            outcat_sb = singles.tile([128, H // 2, N], bf16)
            r_dram = dram.tile([H, 1024], fp32)

            pav_tiles = {}

            def emit_av(hp, ct, ets):
                h0 = 2 * hp
                for h, et in ((h0, ets[0]), (h0 + 1, ets[1])):
                    if ct == 0:
                        pav_tiles[h % 2] = ps_av.tile(
                            [128, N], fp32, tag="av", name=f"pav{hp}_{h % 2}"
                        )
                    pav = pav_tiles[h % 2]
                    for nb in range(2):
                        nc.tensor.matmul(
                            pav[:, nb * 512 : (nb + 1) * 512],
                            v_sb[:, ct, h, :],
                            et[:, nb * 512 : (nb + 1) * 512],
                            start=(ct == 0),
                            stop=(ct == NT - 1),
                        )
                    if ct == NT - 1:
                        emit_normalize(h, pav)

            def emit_normalize(h, pav):
                # Evacuate pav to SBUF promptly so the psum slot frees for the
                # next pair. Full 128-partition copies: partial-partition DVE
                # ops run at proportionally reduced rate, so copying the
                # garbage rows too is faster. Even head on ACT, odd on DVE.
                qrow = (h % 2) * 64
                srow = D if h % 2 == 0 else 0
                av_sb = temps.tile([128, 1024], fp32, tag="avs", name=f"avs{h}")
                if h % 2 == 0:
                    nc.scalar.copy(out=av_sb, in_=pav)
                else:
                    nc.vector.tensor_copy(av_sb, pav)
                # 1/S: S sits on one partition; reshape S to [128, 8] via
                # SBUF->SBUF DMA so the reciprocal is partition-parallel,
                # then a DRAM round trip broadcasts 1/S over 64 partitions.
                s128 = temps.tile([128, 8], fp32, tag="s128")
                nc.sync.dma_start(out=s128, in_=av_sb[srow : srow + 1, :])
                r128 = temps.tile([128, 8], fp32, tag="r128")
                nc.vector.reciprocal(out=r128, in_=s128)
                nc.sync.dma_start(out=r_dram[h : h + 1, :], in_=r128)
                rb_sb = temps.tile([128, 1024], fp32, tag="rb", name=f"rb{h}")
                rb_q = nc.gpsimd if h % 2 == 0 else nc.sync
                rb_q.dma_start(
                    out=rb_sb[qrow : qrow + 64, :],
                    in_=bass.AP(
                        tensor=r_dram.tensor,
                        offset=r_dram.offset + h * 1024,
                        ap=[[0, 64], [1, 1024]],
                    ),
                )
                mul_eng = nc.vector if h % 2 == 0 else nc.gpsimd
                mul_eng.tensor_mul(
                    outcat_sb[qrow : qrow + 64, h // 2, :],
                    av_sb[qrow : qrow + 64, :],
                    rb_sb[qrow : qrow + 64, :],
                )

            pending = []
            for hp in range(H // 2):
                h0, h1 = 2 * hp, 2 * hp + 1
                if hp + 2 < H // 2:
                    load_krr(hp + 2)  # prefetch 2 pairs ahead
                for ct in range(NT):
                    kr0 = krr_all[:, h0, ct * 128 : (ct + 1) * 128]
                    kr1 = krr_all[:, h1, ct * 128 : (ct + 1) * 128]
                    # pd tiles span 2 psum banks each: a single [128, 1024]
                    # exp instruction then reads both banks interleaved at 2x
                    # the single-bank rate (psum reads are the exp bottleneck).
                    pd0 = ps_sm.tile([128, N], fp32, tag="sm", name=f"pd0_{hp}_{ct}")
                    pd1 = ps_sm.tile([128, N], fp32, tag="sm", name=f"pd1_{hp}_{ct}")
                    for nb in range(2):
                        nc.tensor.matmul(
                            pd1[:, nb * 512 : (nb + 1) * 512],
                            kr1,
                            qT_sb[:, hp, nb * 512 : (nb + 1) * 512],
                            start=True,
                            stop=True,
                        )
                    for nb in range(2):
                        nc.tensor.matmul(
                            pd0[:, nb * 512 : (nb + 1) * 512],
                            kr0,
                            qT_sb[:, hp, nb * 512 : (nb + 1) * 512],
                            start=True,
                            stop=True,
                        )
                    # exp: even head exact on ACT (~1.11us), odd head via
                    # the DVE Schraudolph bit trick (~1.02us); one engine
                    # alone would be the attention bottleneck
                    et0 = exps.tile([128, N], bf16, tag="exp", name=f"e0_{hp}_{ct}")
                    et1i = exps.tile([128, N], i16, tag="exps", name=f"e1_{hp}_{ct}")
                    nc.scalar.activation(out=et0, in_=pd0, func=AF.Exp)
                    nc.vector.tensor_scalar(
                        out=et1i,
                        in0=pd1,
                        scalar1=SCH_A,
                        scalar2=SCH_B,
                        op0=ALU.mult,
                        op1=ALU.add,
                    )
                    pending.append((hp, ct, (et0, et1i.bitcast(bf16))))
                    if len(pending) > 1:
                        emit_av(*pending.pop(0))
            while pending:
                emit_av(*pending.pop(0))


            for i in range(24):
                pw = ps_av.tile([128, 512], fp32, tag="av", name=f"fill{i}")
                nc.tensor.matmul(pw, warm[:, 0:128], warm, start=True, stop=True)

            # ---- phase 5: projection + LayerNorm + residual
            for m in range(NT):
                pool_m = ps_av if m % 2 == 0 else ps_sm
                py = pool_m.tile(
                    [128, 512], fp32, tag="av" if m % 2 == 0 else "sm",
                    name=f"py{m}",
                )
                for p in range(H // 2):
                    nc.tensor.matmul(
                        py,
                        outcat_sb[:, p, m * 128 : (m + 1) * 128],
                        wout_sb[:, p, :],
                        start=(p == 0),
                        stop=(p == H // 2 - 1),
                    )
                if bb_sb is not None:
                    nc.vector.tensor_add(py, py, bb_sb)
                stats = lnp.tile([128, 6], fp32, tag="stats")
                nc.vector.bn_stats(out=stats, in_=py)
                mv = lnp.tile([128, 2], fp32, tag="mv")
                nc.vector.bn_aggr(out=mv, in_=stats)
                # rstd = exp(-0.5 * ln(var + eps)) -- stays in the exp/ln set
                lnvar = lnp.tile([128, 1], fp32, tag="lnvar")
                nc.scalar.activation(
                    out=lnvar, in_=mv[:, 1:2], func=AF.Ln, bias=eps_sb
                )
                rstd = lnp.tile([128, 1], fp32, tag="rstd")
                nc.scalar.activation(out=rstd, in_=lnvar, func=AF.Exp, scale=-0.5)
                nmr = lnp.tile([128, 1], fp32, tag="nmr")
                nc.vector.tensor_scalar(
                    out=nmr,
                    in0=mv[:, 0:1],
                    scalar1=rstd[:, 0:1],
                    scalar2=-1.0,
                    op0=ALU.mult,
                    op1=ALU.mult,
                )
                fin = temps.tile([128, 512], fp32, tag="fin")
                if trivial_gamma:
                    # xhat = py*rstd + (-mu*rstd) on ACT (idle during proj;
                    # the DVE chain was the proj-phase critical path)
                    xh0 = temps.tile([128, 512], fp32, tag="xh")
                    nc.scalar.activation(
                        out=xh0,
                        in_=py,
                        func=AF.Identity,
                        bias=nmr[:, 0:1],
                        scale=rstd[:, 0:1],
                    )
                    # residual add split across DVE and gpsimd: either
                    # engine alone is the proj-phase critical path
                    nc.vector.tensor_add(
                        fin[:, 0:192], xh0[:, 0:192], x_sb[:, m, 0:192]
                    )
                    nc.gpsimd.tensor_add(
                        fin[:, 192:512], xh0[:, 192:512], x_sb[:, m, 192:512]
                    )
                    if bb2_sb is not None:
                        nc.gpsimd.tensor_add(fin, fin, bb2_sb)
                else:
                    xh = temps.tile([128, 512], fp32, tag="xh")
                    nc.vector.tensor_scalar(
                        out=xh,
                        in0=py,
                        scalar1=rstd[:, 0:1],
                        scalar2=nmr[:, 0:1],
                        op0=ALU.mult,
                        op1=ALU.add,
                    )
                    nc.vector.tensor_mul(xh, xh, gb_sb)
                    nc.vector.tensor_add(fin, xh, x_sb[:, m, :])
                    if bb2_sb is not None:
                        nc.vector.tensor_add(fin, fin, bb2_sb)
                nc.sync.dma_start(out=out_d.ap()[m * 128 : (m + 1) * 128, :], in_=fin)

    return nc


def _get_program(trivial_bias, trivial_gamma, trivial_beta):
    key = (trivial_bias, trivial_gamma, trivial_beta)
    if key not in _cache:
        _cache[key] = _build(*key)
    return _cache[key]


def kernel(x, w_qkv, w_out, b_out, ln_g, ln_b):
    global last_results
    from concourse import bass_utils

    x = np.ascontiguousarray(np.asarray(x, dtype=np.float32))
    w_qkv = np.ascontiguousarray(np.asarray(w_qkv, dtype=np.float32))
    w_out = np.ascontiguousarray(np.asarray(w_out, dtype=np.float32))
    b_out = np.asarray(b_out, dtype=np.float32).reshape(1, DIM)
    ln_g = np.asarray(ln_g, dtype=np.float32).reshape(1, DIM)
    ln_b = np.asarray(ln_b, dtype=np.float32).reshape(1, DIM)

    nc = _get_program(
        not np.any(b_out), bool(np.all(ln_g == 1.0)), not np.any(ln_b)
    )
    if not getattr(nc, "_waits_split", False):
        _split_sync_waits(nc)
        nc._waits_split = True

    in_maps = [
        {
            "x": np.ascontiguousarray(x[c]),
            "w_qkv": w_qkv,
            "w_out": w_out,
            "b_out": b_out,
            "ln_g": ln_g,
            "ln_b": ln_b,
        }
        for c in range(N_CORES)
    ]
    trace = bool(int(os.environ.get("BENCH_TRACE", "0")))
    res = bass_utils.run_bass_kernel_spmd(
        nc, in_maps, core_ids=list(range(N_CORES)), trace=trace
    )
    last_results = res
    return np.stack([res.results[c]["out"] for c in range(N_CORES)], axis=0)


# revision 21
# speedup vs baseline: 1.1444x; 1.1444x over previous
"""Trainium2 Bass kernel for the fused attention block:

    qkv = x @ w_qkv ; q,k,v split; heads; dots = q @ k.reshape(bh, D, n)
    attn = softmax(dots); out = attn @ v; merge heads; out = out @ w_out + b_out
    out = LayerNorm(out) * ln_g + ln_b; return out + x

Sharding: data-parallel over batch b (8 batches -> 8 NeuronCores, weights
replicated). Each core runs an identical program on its own batch slice.

Key layout choices (per core, N=1024 seq, DIM=512, H=8 heads, D=64):
  - xT [512, 1024] via matmul-transposes (x chunk stationary, identity
    moving): a REGULAR full-array matmul registers as HAM activity, so
    phase 1 doubles as the PE clock-gate warmup. x is DMA'd in 8 row
    chunks so the transposes chase the DMA.
  - qT [512, 1024]  = matmul(lhsT=w_q, rhs=xT), rounded to bf16 at psum
    evacuation (the dots operands are bf16 to halve the k DMA round trip).
  - k is round-tripped through a DRAM scratch (bf16) so the faithful
    k.reshape(D, n) ("k_r") can be gathered as [64, 1024] with d on
    partitions; the other parity's 64 rows stay zero so the K=128 dots
    contraction nulls the other head's rows of the shared qT pair tile.
    (Half-array K=64 matmuls are NOT used: they neither register as HAM
    activity nor run at the warm clock.)
  - exp: both heads on ACT as single [128, 1024] instructions -- psum
    reads bottleneck exp, and a 2-psum-bank tile reads twice as fast as
    a 1-bank tile, so never split exp below 1024 columns.
  - out_hT[e, i] += matmul(lhsT=[v|ones] block, rhs=expT); the ones column
    makes the same accumulation chain produce the softmax denominator S[i].
    Garbage in the unused output partitions is never read.
  - normalize with a partition-parallel reciprocal + DRAM-broadcast of 1/S;
    pav evacuations copy the full 128 partitions (partial-partition DVE ops
    run at proportionally reduced rate).
  - final = matmul(lhsT=out_catT, rhs=w_out) -> LN (bn_stats/bn_aggr,
    rsqrt via exp(-0.5*ln(var+eps)) to stay in one ACT table set) +
    residual, with the residual add split across DVE and gpsimd.
"""

import os
import numpy as np

B, N, DIM = 8, 1024, 512
H, D = 8, 64
LN_EPS = 1e-5
N_CORES = 8

# Schraudolph exp in bf16: bitcast(int16(x*SCH_A + SCH_B)) ~= exp(x).
# The -5.5 recentres the piecewise-linear 2^frac error to +-3.2%.
SCH_A = 128.0 * np.log2(np.e)
SCH_B = 127.0 * 128.0 - 5.5

_cache = {}
last_results = None


MAX_WAITS = 1


def _split_sync_waits(nc, limit=MAX_WAITS):
    """This walrus build rejects instructions carrying more than `limit`
    sem-wait commands ("Too many sync wait commands"). Move excess waits
    onto same-engine NOPs inserted immediately before the instruction
    (per-engine program order is list order, so semantics are identical)."""
    import concourse.mybir as mybir

    for fn in nc.m.functions:
        for bb in fn.blocks:
            out = []
            for ins in bb.instructions:
                si = getattr(ins, "sync_info", None)
                keep = 0 if type(ins).__name__ in ("InstISA", "InstDrain") else limit
                if si is not None and si.on_wait and len(si.on_wait) > keep:
                    waits = list(si.on_wait)
                    si.on_wait = waits[len(waits) - keep :] if keep else []
                    extra = waits[: len(waits) - keep]
                    for i in range(0, len(extra), limit):
                        out.append(
                            mybir.InstNoOp(
                                name=f"{ins.name}_w{i}",
                                engine=ins.engine,
                                debug=ins.debug,
                                bass_nofuse=True,
                                sync_info=mybir.SyncInfo(
                                    on_wait=extra[i : i + limit], on_update=[]
                                ),
                            )
                        )
                out.append(ins)
            bb.instructions = out


def _patch_sem_clear():
    """EVENT_SEMAPHORE_RANGE_CLEAR with a large sem range fails walrus
    codegen ("ISA wrong length"); chunk the tail sem clear into <=48-sem
    ranges (the size known to compile)."""
    import concourse.bass as bass
    from concourse.bass import SemaphoreHandle

    if getattr(bass.Bass, "_sem_clear_patched", False):
        return
    from concourse.bass import compact_to_ranges

    def clear_and_free_semaphores(self, sems):
        if not sems:
            return
        sem_nums = [s.num if isinstance(s, SemaphoreHandle) else s for s in sems]
        for sem_range in compact_to_ranges(sem_nums):
            for lo in range(sem_range.start, sem_range.stop, 48):
                sub = range(lo, min(lo + 48, sem_range.stop))
                assert self._state.free_isdisjoint(sub)
                self.gpsimd.dma_reset(sub)
                self.gpsimd.sem_clear(sub)
        self._state.prepend_free_semaphores(sem_nums)
        for poison_set in self._tile_sem_poison_stack:
            poison_set.update(sem_nums)

    bass.Bass.clear_and_free_semaphores = clear_and_free_semaphores
    bass.Bass._sem_clear_patched = True

    import concourse.tile as tile
    from concourse.vector_clock import ScopedClock

    def _drain_and_barrier(self, tick_clock, wait_clock):
        drain_inst = self.nc.sync.drain()
        wait_clock.add_sem_waits(
            drain_inst.ins, ScopedClock({None: tick_clock.global_clock})
        )
        self.nc.all_engine_barrier()
        popped = self.nc._tile_sem_poison_stack.pop()
        assert popped is self._sem_poison
        self.nc.clear_and_free_semaphores(list(self.sems.allocated().values()))

    tile.TileContext._drain_and_barrier = _drain_and_barrier


def _build(trivial_bias: bool, trivial_gamma: bool, trivial_beta: bool):
    import concourse.bass as bass
    import concourse.mybir as mybir
    import concourse.tile as tile
    from concourse.masks import make_identity

    _patch_sem_clear()

    fp32 = mybir.dt.float32
    fp32r = mybir.dt.float32r
    bf16 = mybir.dt.bfloat16
    i16 = mybir.dt.int16
    AF = mybir.ActivationFunctionType
    ALU = mybir.AluOpType

    nc = bass.Bass("TRN2", target_bir_lowering=False, debug=False)

    x_d = nc.dram_tensor("x", [N, DIM], fp32, kind="ExternalInput")
    wqkv_d = nc.dram_tensor("w_qkv", [DIM, 3 * DIM], fp32r, kind="ExternalInput")
    wout_d = nc.dram_tensor("w_out", [DIM, DIM], fp32, kind="ExternalInput")
    bout_d = nc.dram_tensor("b_out", [1, DIM], fp32, kind="ExternalInput")
    lng_d = nc.dram_tensor("ln_g", [1, DIM], fp32, kind="ExternalInput")
    lnb_d = nc.dram_tensor("ln_b", [1, DIM], fp32, kind="ExternalInput")
    out_d = nc.dram_tensor("out", [N, DIM], fp32, kind="ExternalOutput")

    NT = N // 128      # 8 i-tiles (also c-tiles)
    KC = DIM // 128    # 4 contraction chunks

    with tile.TileContext(nc) as tc:
        import contextlib

        ctx = contextlib.ExitStack()
        with ctx:
            singles = ctx.enter_context(tc.tile_pool(name="singles", bufs=1))
            dram = ctx.enter_context(tc.tile_pool(name="dram", bufs=1, space="DRAM"))
            # 2x [128,1024] general psum slots + 2x [128,1024] AV slots
            ps_sm = ctx.enter_context(tc.tile_pool(name="ps_sm", bufs=2, space="PSUM"))
            ps_av = ctx.enter_context(tc.tile_pool(name="ps_av", bufs=2, space="PSUM"))
            temps = ctx.enter_context(tc.tile_pool(name="temps", bufs=4))
            exps = ctx.enter_context(tc.tile_pool(name="exps", bufs=4))
            lnp = ctx.enter_context(tc.tile_pool(name="lnp", bufs=6))

            # ---- constants
            ident_bf = singles.tile([128, 128], bf16)
            make_identity(nc, ident_bf)
            eps_sb = singles.tile([128, 1], fp32)
            nc.vector.memset(eps_sb, LN_EPS)

            # ---- input loads. x first (needed from ~1.5us), chunked per
            # m-tile so ph1 transposes can chase the DMA; then wq (needed
            # ~8us), wk, wv. Small/late tensors go on the gpsimd queue.
            x_sb = singles.tile([128, NT, DIM], fp32)  # x[128*m + p, c]
            xb_sb = singles.tile([128, NT, DIM], bf16)  # bf16 copy for ph1
            for m in range(NT):
                nc.gpsimd.dma_start(
                    out=xb_sb[:, m, :], in_=x_d.ap()[m * 128 : (m + 1) * 128, :]
                )
            wq_sb = singles.tile([128, KC, DIM], fp32r)
            wk_sb = singles.tile([128, KC, DIM], fp32r)
            wv_sb = singles.tile([128, KC, DIM], fp32r)
            for w_sb, lo in ((wq_sb, 0), (wk_sb, DIM), (wv_sb, 2 * DIM)):
                for kc in range(KC):
                    nc.sync.dma_start(
                        out=w_sb[:, kc, :],
                        in_=wqkv_d.ap()[kc * 128 : (kc + 1) * 128, lo : lo + DIM],
                    )
            # fp32 x for the residual: needed only by the projection phase,
            # so it loads after everything else on the sync queue
            for m in range(NT):
                nc.sync.dma_start(
                    out=x_sb[:, m, :], in_=x_d.ap()[m * 128 : (m + 1) * 128, :]
                )
            # w_out stored per head PAIR ([128, 4, 512]) so the projection
            # contracts K=128 (full array).
            wout_sb = singles.tile([128, H // 2, DIM], bf16)
            nc.gpsimd.dma_start(
                out=wout_sb, in_=wout_d.ap().rearrange("(p r) f -> r p f", r=128)
            )

            bb_sb = gb_sb = bb2_sb = None
            if not trivial_bias:
                bb_sb = singles.tile([128, DIM], fp32)
                nc.gpsimd.dma_start(
                    out=bb_sb,
                    in_=bass.AP(tensor=bout_d, offset=0, ap=[[0, 128], [1, DIM]]),
                )
            if not trivial_gamma:
                gb_sb = singles.tile([128, DIM], fp32)
                nc.gpsimd.dma_start(
                    out=gb_sb,
                    in_=bass.AP(tensor=lng_d, offset=0, ap=[[0, 128], [1, DIM]]),
                )
            if not trivial_beta:
                bb2_sb = singles.tile([128, DIM], fp32)
                nc.gpsimd.dma_start(
                    out=bb2_sb,
                    in_=bass.AP(tensor=lnb_d, offset=0, ap=[[0, 128], [1, DIM]]),
                )

            # ---- v blocks: [128, 128] lhsT per (tile, head). Even head ->
            # v in cols 0:64 + ones col 64 (AV output in psum rows 0:64, S in
            # row 64); odd head -> v in cols 64:128 + ones col 0 (output rows
            # 64:128, S row 0). The never-written filler columns produce
            # garbage only in psum rows that are never read, so only the ones
            # columns need initialising -- but memset everything once on the
            # (otherwise idle) gpsimd engine to keep NaNs out of the array.
            v_sb = singles.tile([128, NT, H, 128], bf16)
            nc.gpsimd.memset(v_sb, 0.0)
            v_par = v_sb.rearrange("p m (h2 par) c -> p m h2 par c", par=2)
            nc.gpsimd.memset(v_par[:, :, :, 0, D : D + 1], 1.0)
            nc.gpsimd.memset(v_par[:, :, :, 1, 0:1], 1.0)

            warm = singles.tile([128, 512], fp32r)
            nc.vector.memset(warm.bitcast(fp32), 1.0)
            for i in range(8):
                pw = ps_av.tile([128, 512], fp32, tag="av", name=f"pw{i}")
                nc.tensor.matmul(pw, warm[:, 0:128], warm, start=True, stop=True)

            # ---- phase 1: xT[k, i] via matmul-transposes, chasing the x
            # DMA: out = x_chunk.T @ I. A REGULAR full-array matmul (lhsT =
            # x chunk as stationary, identity as moving) -- unlike
            # transpose-mode this registers as HAM activity, so it doubles
            # as the clock-gate warmup and needs no junk-matmul fillers.
            xT_sb = singles.tile([128, KC, N], fp32r)
            for mp in range(NT // 2):
                pt = ps_sm.tile([128, 1024], fp32, tag="sm", name=f"pt{mp}")
                ptv = pt.rearrange("p (mh kc c) -> p mh kc c", mh=2, kc=KC)
                for mh in range(2):
                    m = 2 * mp + mh
                    for kc in range(KC):
                        nc.tensor.matmul(
                            ptv[:, mh, kc, :],
                            xb_sb[:, m, kc * 128 : (kc + 1) * 128],
                            ident_bf,
                            start=True,
                            stop=True,
                        )
                nc.scalar.copy(
                    out=xT_sb[:, :, 2 * mp * 128 : (2 * mp + 2) * 128].rearrange(
                        "p kc (mh c) -> p kc mh c", mh=2
                    ),
                    in_=ptv.rearrange("p mh kc c -> p kc mh c"),
                )

            # ---- phase 2: qT[qd, i], two heads per tile (M=128, full array).
            # qT is rounded to bf16 at psum evacuation: the row-tiled dots
            # pair can only stream both rhs operands concurrently at 2 bytes
            # per element (two fp32r streams saturate the moving-operand bus
            # and the pair runs at half rate).
            qT_sb = singles.tile([128, KC, N], bf16)
            for m in range(KC):
                pq = [
                    ps_sm.tile([128, 512], fp32, tag="sm", name=f"pq{m}_{nb}")
                    for nb in range(2)
                ]
                for kc in range(KC):
                    for nb in range(2):
                        nc.tensor.matmul(
                            pq[nb],
                            wq_sb[:, kc, m * 128 : (m + 1) * 128],
                            xT_sb[:, kc, nb * 512 : (nb + 1) * 512],
                            start=(kc == 0),
                            stop=(kc == KC - 1),
                        )
                for nb in range(2):
                    nc.vector.tensor_copy(
                        qT_sb[:, m, nb * 512 : (nb + 1) * 512], pq[nb]
                    )

            # ---- phase 3a: k natural -> DRAM scratch (so the faithful
            # k_r reshape can be gathered with d on partitions).
            k_dram = dram.tile([N, DIM], bf16)
            for m in range(NT):
                pk = ps_sm.tile([128, DIM], fp32, tag="sm", name=f"pk{m}")
                for kc in range(KC):
                    nc.tensor.matmul(
                        pk,
                        xT_sb[:, kc, m * 128 : (m + 1) * 128],
                        wk_sb[:, kc, :],
                        start=(kc == 0),
                        stop=(kc == KC - 1),
                    )
                ktmp = temps.tile([128, DIM], bf16, tag="ktmp")
                nc.vector.tensor_copy(ktmp, pk)
                nc.sync.dma_start(out=k_dram[m * 128 : (m + 1) * 128, :], in_=ktmp)

            # krr gathers: head h's k_r sits at its parity rows ((h%2)*64);
            # the other 64 rows stay ZERO so the K=128 dots contraction
            # nulls the other head's rows of the shared qT pair tile.
            krr_all = singles.tile([128, H, N], bf16)
            nc.gpsimd.memset(krr_all, 0.0)

            def load_krr(hp):
                for hh in (2 * hp, 2 * hp + 1):
                    r0 = (hh % 2) * 64
                    nc.gpsimd.dma_start(
                        out=krr_all[r0 : r0 + 64, hh, :].rearrange(
                            "p (s c) -> p s c", s=16
                        ),
                        in_=bass.AP(
                            tensor=k_dram.tensor,
                            offset=k_dram.offset + hh * 64,
                            ap=[[16 * DIM, 64], [DIM, 16], [1, 64]],
                        ),
                    )

            load_krr(0)
            load_krr(1)

            # ---- phase 3b: v half
            for m in range(NT):
                pvv = ps_sm.tile([128, DIM], fp32, tag="sm", name=f"pvv{m}")
                for kc in range(KC):
                    nc.tensor.matmul(
                        pvv,
                        xT_sb[:, kc, m * 128 : (m + 1) * 128],
                        wv_sb[:, kc, :],
                        start=(kc == 0),
                        stop=(kc == KC - 1),
                    )
                vv = v_sb[:, m, :, :].rearrange("p (h2 par) c -> p h2 par c", par=2)
                pv = pvv.rearrange("p (h2 par e) -> p h2 par e", h2=4, par=2)
                nc.vector.tensor_copy(vv[:, :, 0, 0:64], pv[:, :, 0, :])
                nc.vector.tensor_copy(vv[:, :, 1, 64:128], pv[:, :, 1, :])

            # ---- phase 4: attention.
            # Per (pair, ct): row-tiled dots (2 concurrent K=64 matmuls per
            # nb slot), exp split ACT/DVE, then the previous ct's AV pair.
            # PSUM: 4x dots [128,512] (ps_sm) + 2x pav [128,1024] (ps_av)
            # fills all 8 banks, so there are no fillers inside this phase.
            outcat_sb = singles.tile([128, H // 2, N], bf16)
            r_dram = dram.tile([H, 1024], fp32)

            pav_tiles = {}

            def emit_av(hp, ct, ets):
                h0 = 2 * hp
                for h, et in ((h0, ets[0]), (h0 + 1, ets[1])):
                    if ct == 0:
                        pav_tiles[h % 2] = ps_av.tile(
                            [128, N], fp32, tag="av", name=f"pav{hp}_{h % 2}"
                        )
                    pav = pav_tiles[h % 2]
                    for nb in range(2):
                        nc.tensor.matmul(
                            pav[:, nb * 512 : (nb + 1) * 512],
                            v_sb[:, ct, h, :],
                            et[:, nb * 512 : (nb + 1) * 512],
                            start=(ct == 0),
                            stop=(ct == NT - 1),
                        )
                    if ct == NT - 1:
                        emit_normalize(h, pav)

            def emit_normalize(h, pav):
                # Evacuate pav to SBUF promptly so the psum slot frees for the
                # next pair. Full 128-partition copies: partial-partition DVE
                # ops run at proportionally reduced rate, so copying the
                # garbage rows too is faster. Even head on ACT, odd on DVE.
                qrow = (h % 2) * 64
                srow = D if h % 2 == 0 else 0
                av_sb = temps.tile([128, 1024], fp32, tag="avs", name=f"avs{h}")
                if h % 2 == 0:
                    nc.scalar.copy(out=av_sb, in_=pav)
                else:
                    nc.vector.tensor_copy(av_sb, pav)
                # 1/S: S sits on one partition; reshape S to [128, 8] via
                # SBUF->SBUF DMA so the reciprocal is partition-parallel,
                # then a DRAM round trip broadcasts 1/S over 64 partitions.
                s128 = temps.tile([128, 8], fp32, tag="s128")
                nc.sync.dma_start(out=s128, in_=av_sb[srow : srow + 1, :])
                r128 = temps.tile([128, 8], fp32, tag="r128")
                nc.vector.reciprocal(out=r128, in_=s128)
                nc.sync.dma_start(out=r_dram[h : h + 1, :], in_=r128)
                rb_sb = temps.tile([128, 1024], fp32, tag="rb", name=f"rb{h}")
                rb_q = nc.gpsimd if h % 2 == 0 else nc.sync
                rb_q.dma_start(
                    out=rb_sb[qrow : qrow + 64, :],
                    in_=bass.AP(
                        tensor=r_dram.tensor,
                        offset=r_dram.offset + h * 1024,
                        ap=[[0, 64], [1, 1024]],
                    ),
                )
                mul_eng = nc.vector if h % 2 == 0 else nc.gpsimd
                mul_eng.tensor_mul(
                    outcat_sb[qrow : qrow + 64, h // 2, :],
                    av_sb[qrow : qrow + 64, :],
                    rb_sb[qrow : qrow + 64, :],
                )

            pending = []
            for hp in range(H // 2):
                h0, h1 = 2 * hp, 2 * hp + 1
                if hp + 2 < H // 2:
                    load_krr(hp + 2)  # prefetch 2 pairs ahead
                for ct in range(NT):
                    kr0 = krr_all[:, h0, ct * 128 : (ct + 1) * 128]
                    kr1 = krr_all[:, h1, ct * 128 : (ct + 1) * 128]
                    # pd tiles span 2 psum banks each: a single [128, 1024]
                    # exp instruction then reads both banks interleaved at 2x
                    # the single-bank rate (psum reads are the exp bottleneck).
                    pd0 = ps_sm.tile([128, N], fp32, tag="sm", name=f"pd0_{hp}_{ct}")
                    pd1 = ps_sm.tile([128, N], fp32, tag="sm", name=f"pd1_{hp}_{ct}")
                    for nb in range(2):
                        nc.tensor.matmul(
                            pd1[:, nb * 512 : (nb + 1) * 512],
                            kr1,
                            qT_sb[:, hp, nb * 512 : (nb + 1) * 512],
                            start=True,
                            stop=True,
                        )
                    for nb in range(2):
                        nc.tensor.matmul(
                            pd0[:, nb * 512 : (nb + 1) * 512],
                            kr0,
                            qT_sb[:, hp, nb * 512 : (nb + 1) * 512],
                            start=True,
                            stop=True,
                        )
                    # exp: even head exact on ACT (~1.11us), odd head via
                    # the DVE Schraudolph bit trick (~1.02us); one engine
                    # alone would be the attention bottleneck
                    et0 = exps.tile([128, N], bf16, tag="exp", name=f"e0_{hp}_{ct}")
                    et1i = exps.tile([128, N], i16, tag="exps", name=f"e1_{hp}_{ct}")
                    nc.scalar.activation(out=et0, in_=pd0, func=AF.Exp)
                    nc.vector.tensor_scalar(
                        out=et1i,
                        in0=pd1,
                        scalar1=SCH_A,
                        scalar2=SCH_B,
                        op0=ALU.mult,
                        op1=ALU.add,
                    )
                    pending.append((hp, ct, (et0, et1i.bitcast(bf16))))
                    if len(pending) > 1:
                        emit_av(*pending.pop(0))
            while pending:
                emit_av(*pending.pop(0))


            for i in range(8):
                pw = ps_av.tile([128, 512], fp32, tag="av", name=f"fill{i}")
                nc.tensor.matmul(pw, warm[:, 0:128], warm, start=True, stop=True)

            # ---- phase 5: projection + LayerNorm + residual
            for m in range(NT):
                pool_m = ps_av if m % 2 == 0 else ps_sm
                py = pool_m.tile(
                    [128, 512], fp32, tag="av" if m % 2 == 0 else "sm",
                    name=f"py{m}",
                )
                for p in range(H // 2):
                    nc.tensor.matmul(
                        py,
                        outcat_sb[:, p, m * 128 : (m + 1) * 128],
                        wout_sb[:, p, :],
                        start=(p == 0),
                        stop=(p == H // 2 - 1),
                    )
                if bb_sb is not None:
                    nc.vector.tensor_add(py, py, bb_sb)
                stats = lnp.tile([128, 6], fp32, tag="stats")
                nc.vector.bn_stats(out=stats, in_=py)
                mv = lnp.tile([128, 2], fp32, tag="mv")
                nc.vector.bn_aggr(out=mv, in_=stats)
                # rstd = exp(-0.5 * ln(var + eps)) -- stays in the exp/ln set
                lnvar = lnp.tile([128, 1], fp32, tag="lnvar")
                nc.scalar.activation(
                    out=lnvar, in_=mv[:, 1:2], func=AF.Ln, bias=eps_sb
                )
                rstd = lnp.tile([128, 1], fp32, tag="rstd")
                nc.scalar.activation(out=rstd, in_=lnvar, func=AF.Exp, scale=-0.5)
                nmr = lnp.tile([128, 1], fp32, tag="nmr")
                nc.vector.tensor_scalar(
                    out=nmr,
                    in0=mv[:, 0:1],
                    scalar1=rstd[:, 0:1],
                    scalar2=-1.0,
                    op0=ALU.mult,
                    op1=ALU.mult,
                )
                fin = temps.tile([128, 512], fp32, tag="fin")
                if trivial_gamma:
                    # xhat = py*rstd + (-mu*rstd) on ACT (idle during proj;
                    # the DVE chain was the proj-phase critical path)
                    xh0 = temps.tile([128, 512], fp32, tag="xh")
                    nc.scalar.activation(
                        out=xh0,
                        in_=py,
                        func=AF.Identity,
                        bias=nmr[:, 0:1],
                        scale=rstd[:, 0:1],
                    )
                    # residual add split across DVE and gpsimd: either
                    # engine alone is the proj-phase critical path
                    nc.vector.tensor_add(
                        fin[:, 0:192], xh0[:, 0:192], x_sb[:, m, 0:192]
                    )
                    nc.gpsimd.tensor_add(
                        fin[:, 192:512], xh0[:, 192:512], x_sb[:, m, 192:512]
                    )
                    if bb2_sb is not None:
                        nc.gpsimd.tensor_add(fin, fin, bb2_sb)
                else:
                    xh = temps.tile([128, 512], fp32, tag="xh")
                    nc.vector.tensor_scalar(
                        out=xh,
                        in0=py,
                        scalar1=rstd[:, 0:1],
                        scalar2=nmr[:, 0:1],
                        op0=ALU.mult,
                        op1=ALU.add,
                    )
                    nc.vector.tensor_mul(xh, xh, gb_sb)
                    nc.vector.tensor_add(fin, xh, x_sb[:, m, :])
                    if bb2_sb is not None:
                        nc.vector.tensor_add(fin, fin, bb2_sb)
                nc.sync.dma_start(out=out_d.ap()[m * 128 : (m + 1) * 128, :], in_=fin)

    return nc


def _get_program(trivial_bias, trivial_gamma, trivial_beta):
    key = (trivial_bias, trivial_gamma, trivial_beta)
    if key not in _cache:
        _cache[key] = _build(*key)
    return _cache[key]


def kernel(x, w_qkv, w_out, b_out, ln_g, ln_b):
    global last_results
    from concourse import bass_utils

    x = np.ascontiguousarray(np.asarray(x, dtype=np.float32))
    w_qkv = np.ascontiguousarray(np.asarray(w_qkv, dtype=np.float32))
    w_out = np.ascontiguousarray(np.asarray(w_out, dtype=np.float32))
    b_out = np.asarray(b_out, dtype=np.float32).reshape(1, DIM)
    ln_g = np.asarray(ln_g, dtype=np.float32).reshape(1, DIM)
    ln_b = np.asarray(ln_b, dtype=np.float32).reshape(1, DIM)

    nc = _get_program(
        not np.any(b_out), bool(np.all(ln_g == 1.0)), not np.any(ln_b)
    )
    if not getattr(nc, "_waits_split", False):
        _split_sync_waits(nc)
        nc._waits_split = True

    in_maps = [
        {
            "x": np.ascontiguousarray(x[c]),
            "w_qkv": w_qkv,
            "w_out": w_out,
            "b_out": b_out,
            "ln_g": ln_g,
            "ln_b": ln_b,
        }
        for c in range(N_CORES)
    ]
    trace = bool(int(os.environ.get("BENCH_TRACE", "0")))
    res = bass_utils.run_bass_kernel_spmd(
        nc, in_maps, core_ids=list(range(N_CORES)), trace=trace
    )
    last_results = res
    return np.stack([res.results[c]["out"] for c in range(N_CORES)], axis=0)
